# revision 1
# baseline (speedup 1.0000x reference)
"""Trainium2 Bass kernel for the PageRank-propagation problem.

out[i] = (1-C) * sum_j P[i,j] * s[j] / rs[j] + C/n
  P = |Bsym - sim|,  Bsym = triu(B,1) + triu(B,1).T,  rs[j] = sum_k P[j,k]

Sharding: rows split across 8 cores (1024 rows each).  Each core receives its
row block with columns ROTATED by r0 so the compiled SPMD program is identical
on every core: rotated column j'' maps to global column (r0 + j'') mod n.
  j'' in [0, 1024)    : diagonal band (per-element triangular select)
  j'' in [1024, 8192) : off-band; host supplies row-slice / transposed
                        col-slice values directly (layout-only transforms)

Inputs are downcast to bf16 on the host (halves HBM traffic — the kernel is
memory-bound; the final error stays at the 1e-4 level set by the bf16 P
representation).  Host packs each compute tile's operands adjacently so every
SBUF tile needs exactly ONE DMA.

Phase 1 (DMA-bound): per tile, D = X - sim on DVE (bf16 2x mode; diagonal
band tiles assembled with gpsimd affine_selects), then P = |D| on ACT (Abs)
into an SBUF-resident bf16 buffer with the row-sum accumulated for free.
AllGather of the per-core row sums (4 KiB).  Phase 2: t = s * recip(rs)
(f32), rotated via dynamic-offset DMAs from a duplicated DRAM copy, broadcast
across partitions with K=1 PE matmuls, downcast to a resident bf16 t-row by
ACT, then ONE fused multiply+row-sum (scalar_tensor_tensor, standard ISA) per
128-row subblock on DVE over the whole 8192-wide row.
"""

import sys

sys.path.insert(0, "/opt/trn_rl_repo")

import numpy as np

N = 8192
NCORES = 8
NB = N // NCORES          # rows per core (1024)
SB = NB // 128            # 128-row subblocks per core (8)
BAND = NB                 # rotated diagonal band width
W = 1024                  # wide streaming chunk
NONBAND = N - BAND        # 7168
NW = NONBAND // W         # 7 non-band chunks per subblock
PW = N // 512             # 16 t-broadcast chunks (matmul free-dim limit 512)
RS_SLOTS = SB + NW        # rs partial slots per subblock (15)
BPW = 2 * BAND + 128      # bandpack row width (2176)
C = 0.15

_built = {}


def _band_off(ri, cj):
    """Column offset of block (ri, cj) inside the bandpack row."""
    return 2 * 128 * cj + (128 if cj > ri else 0)


def _P(P_sba, P_sbb, ri):
    """P row-subblock ri lives in half a/b at local offset."""
    half = P_sba if ri < SB // 2 else P_sbb
    base = (ri % (SB // 2)) * N
    return half, base


def _build():
    if "nc" in _built:
        return _built["nc"]
    import concourse.bass as bass
    import concourse.bacc as bacc
    import concourse.tile as tile
    from concourse import mybir

    dt = mybir.dt
    Alu = mybir.AluOpType
    Act = mybir.ActivationFunctionType

    nc = bacc.Bacc(
        "TRN2", target_bir_lowering=False, debug=False, enable_asserts=False,
        num_devices=NCORES,
    )

    # bsi[i, ci, 0, :] = off-band Bsym chunk; bsi[i, ci, 1, :] = sim chunk
    BSI = nc.dram_tensor("bsi", [NB, NW, 2, W], dt.bfloat16, kind="ExternalInput")
    # per (ri, cj) block: [src|sim] (256 cols) or [bu|bl|sim] on the diagonal
    BP = nc.dram_tensor("bp", [NB, BPW], dt.bfloat16, kind="ExternalInput")
    SV = nc.dram_tensor("sv", [N], dt.float32, kind="ExternalInput")
    OUT = nc.dram_tensor("out", [NB], dt.float32, kind="ExternalOutput")

    with tile.TileContext(nc, num_cores=NCORES) as tc:
        import contextlib

        with contextlib.ExitStack() as ctx:
            constp = ctx.enter_context(tc.tile_pool(name="constp", bufs=1))
            pp = ctx.enter_context(tc.tile_pool(name="pp", bufs=1))
            statp = ctx.enter_context(tc.tile_pool(name="statp", bufs=1))
            dramp = ctx.enter_context(
                tc.tile_pool(name="dramp", bufs=1, space="DRAM")
            )

            # ---- persistent tiles ----
            # two halves keep free-dim byte offsets < 64 KiB (the 2x DVE
            # perf mode does not engage on larger AP offsets)
            P_sba = pp.tile([128, SB * N // 2], dt.bfloat16)
            P_sbb = pp.tile([128, SB * N // 2], dt.bfloat16)
            rs_part = statp.tile([128, SB * RS_SLOTS], dt.float32)
            rs_sb = statp.tile([128, SB], dt.float32)
            y_sb = statp.tile([128, SB], dt.float32)
            o_sb = statp.tile([128, SB], dt.float32)
            s_small = statp.tile([128, N // 128], dt.float32)
            rs_small = statp.tile([128, N // 128], dt.float32)
            rcp_small = statp.tile([128, N // 128], dt.float32)
            t_small = statp.tile([128, N // 128], dt.float32)

            cc_in = dramp.tile([NB], dt.float32)
            cc_out = dramp.tile([N], dt.float32, addr_space="Shared")
            t_dup = dramp.tile([2 * N], dt.bfloat16)
            t_bf = statp.tile([128, N // 128], dt.bfloat16)

            # s in (p, f) layout: global j = p*64 + f
            nc.sync.dma_start(
                out=s_small[:], in_=SV.ap().rearrange("(p f) -> p f", p=128)
            )

            # ---- phase 1: build P (bf16, SBUF) + row-sum partials ----
            with contextlib.ExitStack() as p1:
                bsip = p1.enter_context(tc.tile_pool(name="bsip", bufs=5))
                bpp = p1.enter_context(tc.tile_pool(name="bpp", bufs=3))
                dtmp = p1.enter_context(tc.tile_pool(name="dtmp", bufs=3))
                xdp = p1.enter_context(tc.tile_pool(name="xdp", bufs=2))

                for ri in range(SB):
                    rowslice = slice(ri * 128, (ri + 1) * 128)
                    pbase = ri * N

                    # band: one DMA for the whole 2176-wide packed row
                    bp_t = bpp.tile([128, BPW], dt.bfloat16, tag="bp")
                    nc.sync.dma_start(out=bp_t[:], in_=BP[rowslice, :])
                    P_half, pb = _P(P_sba, P_sbb, ri)
                    for cj in range(SB):
                        off = _band_off(ri, cj)
                        d_t = dtmp.tile([128, 128], dt.bfloat16, tag="dband")
                        if cj == ri:
                            xu = xdp.tile([128, 128], dt.bfloat16, tag="xu")
                            # strict upper from BU: keep where (f - p) > 0
                            nc.gpsimd.affine_select(
                                out=xu[:], in_=bp_t[:, off:off + 128],
                                compare_op=Alu.is_gt,
                                fill=0.0, base=0, channel_multiplier=-1,
                                pattern=[[1, 128]],
                            )
                            xd = xdp.tile([128, 128], dt.bfloat16, tag="xd")
                            # strict lower from BL: keep where (p - f) > 0
                            nc.gpsimd.affine_select(
                                out=xd[:], in_=bp_t[:, off + 128:off + 256],
                                compare_op=Alu.is_gt,
                                fill=0.0, base=0, channel_multiplier=1,
                                pattern=[[-1, 128]],
                            )
                            nc.gpsimd.tensor_tensor(
                                out=xd[:], in0=xd[:], in1=xu[:], op=Alu.add
                            )
                            nc.gpsimd.tensor_tensor(
                                out=d_t[:], in0=xd[:],
                                in1=bp_t[:, off + 256:off + 384],
                                op=Alu.subtract,
                            )
                        else:
                            nc.vector.tensor_tensor(
                                out=d_t[:], in0=bp_t[:, off:off + 128],
                                in1=bp_t[:, off + 128:off + 256],
                                op=Alu.subtract,
                            )
                        slot = ri * RS_SLOTS + cj
                        nc.scalar.activation(
                            out=P_half[:, pb + cj * 128: pb + (cj + 1) * 128],
                            in_=d_t[:], func=Act.Abs,
                            accum_out=rs_part[:, slot: slot + 1],
                        )

                    # non-band: 7 chunks of 1024 columns, one DMA each
                    for ci in range(NW):
                        bsi_t = bsip.tile([128, 2, W], dt.bfloat16, tag="bsi")
                        nc.sync.dma_start(
                            out=bsi_t[:], in_=BSI[rowslice, ci, :, :]
                        )
                        d_t = dtmp.tile([128, W], dt.bfloat16, tag="dwide")
                        nc.vector.tensor_tensor(
                            out=d_t[:], in0=bsi_t[:, 0, :], in1=bsi_t[:, 1, :],
                            op=Alu.subtract,
                        )
                        slot = ri * RS_SLOTS + SB + ci
                        nc.scalar.activation(
                            out=P_half[:, pb + BAND + ci * W: pb + BAND + (ci + 1) * W],
                            in_=d_t[:], func=Act.Abs,
                            accum_out=rs_part[:, slot: slot + 1],
                        )

            # ---- local row sums -> AllGather ----
            nc.vector.tensor_reduce(
                out=rs_sb[:],
                in_=rs_part[:].rearrange("p (a k) -> p a k", a=SB),
                axis=mybir.AxisListType.X, op=Alu.add,
            )
            # cc_in[g] with g = ri*128 + p  <->  rs_sb[p, ri]
            nc.sync.dma_start(
                out=cc_in[:].rearrange("(a p) -> p a", p=128), in_=rs_sb[:]
            )
            nc.gpsimd.collective_compute(
                "AllGather", Alu.bypass,
                replica_groups=[list(range(NCORES))],
                ins=[cc_in[:]], outs=[cc_out[:]],
            )

            # ---- t = s * recip(rs) in global (p, f) layout ----
            nc.sync.dma_start(
                out=rs_small[:], in_=cc_out[:].rearrange("(p f) -> p f", p=128)
            )
            nc.vector.reciprocal(out=rcp_small[:], in_=rs_small[:])
            nc.vector.tensor_tensor(
                out=t_small[:], in0=s_small[:], in1=rcp_small[:], op=Alu.mult
            )
            nc.scalar.activation(out=t_bf[:], in_=t_small[:], func=Act.Copy)
            # duplicated copy for rotated (wrap-around) reads
            nc.sync.dma_start(
                out=t_dup[0:N].rearrange("(p f) -> p f", p=128), in_=t_bf[:]
            )
            nc.sync.dma_start(
                out=t_dup[N:2 * N].rearrange("(p f) -> p f", p=128), in_=t_bf[:]
            )

            r0v = nc.partition_id() * NB

            # ---- phase 2: y = sum_j P[:, j''] * t_rot[j''] ----
            with contextlib.ExitStack() as p2:
                tbp = p2.enter_context(tc.tile_pool(name="tbp", bufs=1))

                # pad keeps tb_full 128B-aligned in SBUF
                tb_pad = tbp.tile([128, 32], dt.bfloat16)
                tb_full = tbp.tile([128, N], dt.bfloat16)

                # rotated t, replicated across partitions by a single
                # 0-stride-partition DMA from the duplicated DRAM copy
                nc.sync.dma_start(
                    out=tb_full[:],
                    in_=t_dup[bass.ds(r0v, N)].partition_broadcast(128),
                )

                # one fused multiply+row-sum per 128-row subblock
                # (scalar_tensor_tensor = standard TensorScalarPtr;
                # tensor_tensor_reduce is a custom-DVE op unsupported here)
                for ri in range(SB):
                    P_half, pb = _P(P_sba, P_sbb, ri)
                    # in-place: P is dead after its single phase-2 read
                    nc.vector.scalar_tensor_tensor(
                        out=P_half[:, pb:pb + N],
                        in0=P_half[:, pb:pb + N],
                        scalar=1.0, in1=tb_full[:],
                        op0=Alu.bypass, op1=Alu.mult,
                        accum_out=y_sb[:, ri: ri + 1],
                    )

            # out = (1-C) * y + C/n
            nc.scalar.activation(
                out=o_sb[:], in_=y_sb[:],
                func=Act.Copy, bias=float(C / N), scale=float(1.0 - C),
            )
            nc.sync.dma_start(
                out=OUT.ap().rearrange("(a p) -> p a", p=128), in_=o_sb[:]
            )

    nc.finalize()
    _built["nc"] = nc
    return nc


def _shard_inputs(B, sim, s):
    """Layout-only host transforms (slice / transpose / concat / pack),
    plus a bf16 downcast (precision choice of the sharding format)."""
    import ml_dtypes

    bf16 = ml_dtypes.bfloat16
    Bh = B.astype(bf16)
    simh = sim.astype(bf16)
    in_maps = []
    for d in range(NCORES):
        r0, r1 = d * NB, (d + 1) * NB
        # off-band, rotated: global cols [r1..N) then [0..r0)
        bm = np.concatenate(
            [Bh[r0:r1, r1:], np.ascontiguousarray(Bh[:r0, r0:r1].T)], axis=1
        )
        sim_nb = np.concatenate([simh[r0:r1, r1:], simh[r0:r1, :r0]], axis=1)
        bsi = np.stack(
            [bm.reshape(NB, NW, W), sim_nb.reshape(NB, NW, W)], axis=2
        )

        bu = Bh[r0:r1, r0:r1]
        bl = bu.T
        sb = simh[r0:r1, r0:r1]
        rows = []
        for ri in range(SB):
            rs = slice(ri * 128, (ri + 1) * 128)
            pieces = []
            for cj in range(SB):
                cs = slice(cj * 128, (cj + 1) * 128)
                if cj == ri:
                    pieces += [bu[rs, cs], bl[rs, cs], sb[rs, cs]]
                elif cj > ri:
                    pieces += [bu[rs, cs], sb[rs, cs]]
                else:
                    pieces += [bl[rs, cs], sb[rs, cs]]
            rows.append(np.concatenate(pieces, axis=1))
        bp = np.concatenate(rows, axis=0)

        in_maps.append({
            "bsi": np.ascontiguousarray(bsi),
            "bp": np.ascontiguousarray(bp),
            "sv": np.ascontiguousarray(s, dtype=np.float32),
        })
    return in_maps


def kernel(B, similarity_matrix, connectivity_scores, _trace=False, _tmpdir=None):
    from concourse import bass_utils

    B = np.asarray(B, dtype=np.float32)
    sim = np.asarray(similarity_matrix, dtype=np.float32)
    s = np.asarray(connectivity_scores, dtype=np.float32)

    nc = _build()
    in_maps = _shard_inputs(B, sim, s)
    res = bass_utils.run_bass_kernel_spmd(
        nc, in_maps, core_ids=list(range(NCORES)), trace=_trace, tmpdir=_tmpdir
    )
    out = np.concatenate([res.results[d]["out"] for d in range(NCORES)])
    if _trace:
        kernel.last_results = res
    return out



# revision 9
# speedup vs baseline: 1.4762x; 1.4762x over previous
"""Trainium2 Bass kernel for the PageRank-propagation problem.

out[i] = (1-C) * sum_j P[i,j] * t[j] + C/n
  P = |Bsym - sim|,  Bsym = triu(B,1) + triu(B,1).T
  t[j] = s[j] / rs[j],  rs[j] = sum_k P[j,k]

Sharding: rows split across 8 cores (1024 rows each).  TRANSPOSED tile
layout: each core stores P^T chunks [128 partitions = global column j,
1024 free = local row i], with global columns ROTATED by r0 so the SPMD
program is identical on every core: program chunk jc covers global
columns (r0 + jc*128 + p) mod n.  Chunks 0..7 are the diagonal band.

Inputs are downcast to fp8 e4m3 on the host (quarter HBM traffic vs
f32; the kernel is memory-bound).  P is materialized in bf16 in SBUF.

Phase 1, per chunk, split across engines to stay under the DMA roofline:
  - PE path:  D = I*X + (-I)*sim into PSUM (fp8 matmuls), ACT Abs
    PSUM->SBUF bf16.  Band chunks take this path after two gpsimd
    affine_selects build the triangular Bsym piece.
  - DVE path: D = X - sim (TT fp8, 1x) into P, then abs in place
    (tensor_scalar abs_max, 4x) or on ACT.
  - rs: PE matmul with a ones vector (partition-dim reduction),
    accumulated across all 64 chunks in PSUM [1, 1024].
AllGather of the per-core row sums (4 KiB -> 32 KiB).  Phase 2:
t = s * recip(rs) computed in a rotated [64,128] layout (contiguous
DMA from a duplicated DRAM copy), PE-transposed to stationary layout
[128, 64]; the GEMV y = P^T.T @ t runs as 128 PE matmuls with t chunks
stationary, accumulating in PSUM [1, 1024]; final affine on ACT.
"""

import sys

sys.path.insert(0, "/opt/trn_rl_repo")

import numpy as np

N = 8192
NCORES = 8
NB = N // NCORES          # rows per core (1024)
CW = 128                  # chunk width (columns per chunk = partitions)
NCH = N // CW             # 64 chunks per core
BCH = NB // CW            # 8 band chunks
OBCH = NCH - BCH          # 56 off-band chunks
G = 4                     # off-band chunks per DMA group
NOG = OBCH // G           # 14 off-band DMA groups
BG = 2                    # band chunks per DMA group
NBG = BCH // BG           # 4 band DMA groups
C = 0.15

# engine split for off-band chunks (tunable)
PE_OB = 14                # off-band chunks subtracted on PE (rest on DVE)
ACT_STEAL = 3             # every ACT_STEAL-th DVE chunk's abs goes to ACT

_built = {}


def _ob_assign():
    """Assignment of the 56 off-band chunks: True = PE path."""
    pe = []
    cnt = 0
    for i in range(OBCH):
        want = (PE_OB * (i + 1)) // OBCH
        pe.append(want > cnt)
        cnt = want
    return pe


def _build():
    if "nc" in _built:
        return _built["nc"]
    import concourse.bass as bass
    import concourse.bacc as bacc
    import concourse.tile as tile
    from concourse import mybir

    dt = mybir.dt
    Alu = mybir.AluOpType
    Act = mybir.ActivationFunctionType

    nc = bacc.Bacc(
        "TRN2", target_bir_lowering=False, debug=False, enable_asserts=False,
        num_devices=NCORES,
    )

    # off-band groups: [group, partition, chunk-in-group * (X | simT)]
    OB = nc.dram_tensor("ob", [NOG, 128, G, 2 * NB], dt.float8e4,
                        kind="ExternalInput")
    # band groups: [group, partition, chunk-in-group * (M1 | M2 | simT)]
    BND = nc.dram_tensor("bnd", [NBG, 128, BG, 3 * NB], dt.float8e4,
                         kind="ExternalInput")
    # s rotated: srot[c, p] = s[(r0 + c*128 + p) % N]
    SROT = nc.dram_tensor("srot", [NCH, CW], dt.float32, kind="ExternalInput")
    OUT = nc.dram_tensor("out", [NB], dt.float32, kind="ExternalOutput")

    pe_ob = _ob_assign()

    with tile.TileContext(nc, num_cores=NCORES) as tc:
        import contextlib

        with contextlib.ExitStack() as ctx:
            pp = ctx.enter_context(tc.tile_pool(name="pp", bufs=1))
            constp = ctx.enter_context(tc.tile_pool(name="constp", bufs=1))
            statp = ctx.enter_context(tc.tile_pool(name="statp", bufs=1))
            dramp = ctx.enter_context(
                tc.tile_pool(name="dramp", bufs=1, space="DRAM")
            )
            rspp = ctx.enter_context(
                tc.tile_pool(name="rspp", bufs=1, space="PSUM")
            )

            # ---- persistent tiles ----
            # P^T in bf16; two halves keep free-dim byte offsets < 64 KiB
            # (DVE 4x/2x perf modes need AP offsets below 64 KiB)
            P_sba = pp.tile([128, (NCH // 2) * NB], dt.bfloat16)
            P_sbb = pp.tile([128, (NCH // 2) * NB], dt.bfloat16)

            id_pos = constp.tile([128, 128], dt.float8e4)
            id_neg = constp.tile([128, 128], dt.float8e4)
            id64 = constp.tile([64, 64], dt.bfloat16)
            ones_bf = constp.tile([128, 1], dt.bfloat16)

            srot_sb = statp.tile([NCH, CW], dt.float32)
            rs_rot = statp.tile([NCH, CW], dt.float32)
            trc = statp.tile([NCH, CW], dt.float32)
            t_rot_bf = statp.tile([NCH, CW], dt.bfloat16)
            t_stat = statp.tile([128, NCH], dt.bfloat16)
            rs_sb = statp.tile([1, NB], dt.float32)
            out_sb = statp.tile([1, NB], dt.float32)

            cc_in = dramp.tile([NB], dt.float32)
            cc_out = dramp.tile([N], dt.float32, addr_space="Shared")
            cc_dup = dramp.tile([2 * N], dt.float32)

            # rs accumulator: [1, 1024] f32 = 2 PSUM banks, pinned all of
            # phase 1
            rs_ps = rspp.tile([1, NB], dt.float32)

            def P_chunk(jc, h0, h1):
                """P slice for chunk jc, free cols [h0:h1) of its 1024."""
                half = P_sba if jc < NCH // 2 else P_sbb
                base = (jc % (NCH // 2)) * NB
                return half[:, base + h0: base + h1]

            # ---- constants ----
            nc.gpsimd.memset(id_pos[:], 0.0)
            nc.gpsimd.affine_select(
                out=id_pos[:], in_=id_pos[:], compare_op=Alu.not_equal,
                fill=1.0, base=0, channel_multiplier=1, pattern=[[-1, 128]],
            )
            nc.gpsimd.memset(id_neg[:], 0.0)
            nc.gpsimd.affine_select(
                out=id_neg[:], in_=id_neg[:], compare_op=Alu.not_equal,
                fill=-1.0, base=0, channel_multiplier=1, pattern=[[-1, 128]],
            )
            nc.gpsimd.memset(id64[:], 0.0)
            nc.gpsimd.affine_select(
                out=id64[:], in_=id64[:], compare_op=Alu.not_equal,
                fill=1.0, base=0, channel_multiplier=1, pattern=[[-1, 64]],
            )
            nc.vector.memset(ones_bf[:], 1.0)
            nc.sync.dma_start(out=srot_sb[:], in_=SROT.ap())

            rs_n = [0]

            def rs_mm(jc):
                """Accumulate row-sum contribution of chunk jc on PE."""
                rs_n[0] += 1
                first = rs_n[0] == 1
                last = rs_n[0] == NCH
                for h in (0, 512):
                    nc.tensor.matmul(
                        rs_ps[:, h:h + 512], ones_bf[:],
                        P_chunk(jc, h, h + 512),
                        start=first, stop=last,
                    )

            # ---- phase 1 ----
            with contextlib.ExitStack() as p1:
                obp = p1.enter_context(tc.tile_pool(name="obp", bufs=3))
                bndp = p1.enter_context(tc.tile_pool(name="bndp", bufs=2))
                xselp = p1.enter_context(tc.tile_pool(name="xselp", bufs=4))
                psp = p1.enter_context(
                    tc.tile_pool(name="psp", bufs=2, space="PSUM")
                )

                dve_i = [0]

                def do_ob_chunk(jc, xt, a):
                    """Off-band chunk jc from slot a of group tile xt."""
                    x_lo = xt[:, a, 0:NB]
                    s_lo = xt[:, a, NB:2 * NB]
                    if pe_ob[jc - BCH]:
                        ps = psp.tile([128, NB], dt.float32, tag="ps")
                        for h in (0, 512):
                            nc.tensor.matmul(
                                ps[:, h:h + 512], id_pos[:],
                                xt[:, a, h:h + 512],
                                start=True, stop=False,
                            )
                        for h in (0, 512):
                            nc.tensor.matmul(
                                ps[:, h:h + 512], id_neg[:],
                                xt[:, a, NB + h:NB + h + 512],
                                start=False, stop=True,
                            )
                        nc.scalar.activation(
                            out=P_chunk(jc, 0, NB), in_=ps[:], func=Act.Abs,
                        )
                    else:
                        pc = P_chunk(jc, 0, NB)
                        nc.vector.tensor_tensor(
                            out=pc, in0=x_lo, in1=s_lo, op=Alu.subtract,
                        )
                        if dve_i[0] % ACT_STEAL == 0:
                            nc.scalar.activation(out=pc, in_=pc, func=Act.Abs)
                        else:
                            # abs via sign-bit clear (uint16 view, 4x mode)
                            pcu = pc.bitcast(dt.uint16)
                            nc.vector.tensor_scalar(
                                out=pcu, in0=pcu, scalar1=0x7FFF,
                                scalar2=None, op0=Alu.bitwise_and,
                            )
                        dve_i[0] += 1
                    rs_mm(jc)

                def do_band_chunk(jc, bt, a):
                    """Band chunk jc from slot a of band group tile bt."""
                    xs1 = xselp.tile([128, NB], dt.float8e4, tag="xs")
                    xs2 = xselp.tile([128, NB], dt.float8e4, tag="xs")
                    # keep M1 where il < jc*128 + p  (strict upper content)
                    nc.gpsimd.affine_select(
                        out=xs1[:], in_=bt[:, a, 0:NB], compare_op=Alu.is_gt,
                        fill=0.0, base=jc * CW, channel_multiplier=1,
                        pattern=[[-1, NB]],
                    )
                    # keep M2 where il > jc*128 + p  (strict lower content)
                    nc.gpsimd.affine_select(
                        out=xs2[:], in_=bt[:, a, NB:2 * NB],
                        compare_op=Alu.is_gt,
                        fill=0.0, base=-jc * CW, channel_multiplier=-1,
                        pattern=[[1, NB]],
                    )
                    ps = psp.tile([128, NB], dt.float32, tag="ps")
                    for h in (0, 512):
                        nc.tensor.matmul(
                            ps[:, h:h + 512], id_pos[:], xs1[:, h:h + 512],
                            start=True, stop=False,
                        )
                        nc.tensor.matmul(
                            ps[:, h:h + 512], id_pos[:], xs2[:, h:h + 512],
                            start=False, stop=False,
                        )
                    for h in (0, 512):
                        nc.tensor.matmul(
                            ps[:, h:h + 512], id_neg[:],
                            bt[:, a, 2 * NB + h:2 * NB + h + 512],
                            start=False, stop=True,
                        )
                    nc.scalar.activation(
                        out=P_chunk(jc, 0, NB), in_=ps[:], func=Act.Abs,
                    )
                    rs_mm(jc)

                bg_done = 0
                for g in range(NOG):
                    xt = obp.tile([128, G, 2 * NB], dt.float8e4, tag="ob")
                    nc.sync.dma_start(out=xt[:], in_=OB[g])
                    for a in range(G):
                        do_ob_chunk(BCH + g * G + a, xt, a)
                    # interleave band groups through phase 1
                    if g % 3 == 2 and bg_done < NBG:
                        bt = bndp.tile([128, BG, 3 * NB], dt.float8e4,
                                       tag="bnd")
                        nc.sync.dma_start(out=bt[:], in_=BND[bg_done])
                        for a in range(BG):
                            do_band_chunk(bg_done * BG + a, bt, a)
                        bg_done += 1
                while bg_done < NBG:
                    bt = bndp.tile([128, BG, 3 * NB], dt.float8e4, tag="bnd")
                    nc.sync.dma_start(out=bt[:], in_=BND[bg_done])
                    for a in range(BG):
                        do_band_chunk(bg_done * BG + a, bt, a)
                    bg_done += 1

            # ---- row sums -> AllGather (natural order) ----
            nc.scalar.activation(out=rs_sb[:], in_=rs_ps[:], func=Act.Copy)
            nc.sync.dma_start(
                out=cc_in[:].rearrange("(a f) -> a f", a=1), in_=rs_sb[:]
            )
            nc.gpsimd.collective_compute(
                "AllGather", Alu.bypass,
                replica_groups=[list(range(NCORES))],
                ins=[cc_in[:]], outs=[cc_out[:]],
            )
            nc.sync.dma_start(out=cc_dup[0:N], in_=cc_out[:])
            nc.sync.dma_start(out=cc_dup[N:2 * N], in_=cc_out[:])

            r0v = nc.partition_id() * NB

            # ---- t = s * recip(rs), rotated, to stationary layout ----
            nc.sync.dma_start(
                out=rs_rot[:],
                in_=cc_dup[bass.ds(r0v, N)].rearrange("(c p) -> c p", c=NCH),
            )
            nc.vector.reciprocal(out=trc[:], in_=rs_rot[:])
            nc.vector.tensor_tensor(
                out=trc[:], in0=trc[:], in1=srot_sb[:], op=Alu.mult
            )
            nc.scalar.activation(out=t_rot_bf[:], in_=trc[:], func=Act.Copy)

            with contextlib.ExitStack() as p2:
                psp2 = p2.enter_context(
                    tc.tile_pool(name="psp2", bufs=1, space="PSUM")
                )
                tp_ps = psp2.tile([128, NCH], dt.bfloat16)
                y_ps = psp2.tile([1, NB], dt.float32)

                # transpose [64, 128] -> [128, 64]
                nc.tensor.transpose(tp_ps[:], t_rot_bf[:], id64[:])
                nc.vector.tensor_copy(out=t_stat[:], in_=tp_ps[:])

                # ---- phase 2: GEMV y[i] = sum_j P^T[j, i] t[j] on PE ----
                for jc in range(NCH):
                    for h in (0, 512):
                        nc.tensor.matmul(
                            y_ps[:, h:h + 512], t_stat[:, jc:jc + 1],
                            P_chunk(jc, h, h + 512),
                            start=(jc == 0), stop=(jc == NCH - 1),
                        )

                # out = (1-C) * y + C/n
                nc.scalar.activation(
                    out=out_sb[:], in_=y_ps[:], func=Act.Copy,
                    bias=float(C / N), scale=float(1.0 - C),
                )
            nc.sync.dma_start(
                out=OUT.ap().rearrange("(a f) -> a f", a=1), in_=out_sb[:]
            )

    nc.finalize()
    _built["nc"] = nc
    return nc


def _shard_inputs(B, sim, s):
    """Layout-only host transforms (slice / transpose / concat / pack),
    plus an fp8 downcast (precision choice of the sharding format)."""
    import ml_dtypes

    f8 = ml_dtypes.float8_e4m3
    B8 = B.astype(f8)
    sim8 = sim.astype(f8)
    s_ext = np.concatenate([s, s]).astype(np.float32)

    in_maps = []
    for d in range(NCORES):
        r0, r1 = d * NB, (d + 1) * NB

        # off-band chunks, rotated: program chunk jc (8..63) covers global
        # columns j0 = (r0 + jc*128) % N .. +128
        ob = np.empty((NOG, 128, G, 2 * NB), dtype=f8)
        for i in range(OBCH):
            jc = BCH + i
            j0 = (r0 + jc * CW) % N
            if j0 >= r1:
                x = np.ascontiguousarray(B8[r0:r1, j0:j0 + CW].T)
            else:
                x = B8[j0:j0 + CW, r0:r1]
            st = np.ascontiguousarray(sim8[r0:r1, j0:j0 + CW].T)
            g, a = i // G, i % G
            ob[g, :, a, 0:NB] = x
            ob[g, :, a, NB:] = st

        # band chunks: global columns inside [r0, r1)
        Bblk = B8[r0:r1, r0:r1]
        sblk = sim8[r0:r1, r0:r1]
        bnd = np.empty((NBG, 128, BG, 3 * NB), dtype=f8)
        for jc in range(BCH):
            cs = slice(jc * CW, (jc + 1) * CW)
            g, a = jc // BG, jc % BG
            bnd[g, :, a, 0:NB] = np.ascontiguousarray(Bblk[:, cs].T)
            bnd[g, :, a, NB:2 * NB] = Bblk[cs, :]
            bnd[g, :, a, 2 * NB:] = np.ascontiguousarray(sblk[:, cs].T)

        srot = np.ascontiguousarray(
            s_ext[r0:r0 + N].reshape(NCH, CW)
        )

        in_maps.append({
            "ob": ob,
            "bnd": bnd,
            "srot": srot,
        })
    return in_maps


def kernel(B, similarity_matrix, connectivity_scores, _trace=False,
           _tmpdir=None):
    from concourse import bass_utils

    B = np.asarray(B, dtype=np.float32)
    sim = np.asarray(similarity_matrix, dtype=np.float32)
    s = np.asarray(connectivity_scores, dtype=np.float32)

    nc = _build()
    in_maps = _shard_inputs(B, sim, s)
    res = bass_utils.run_bass_kernel_spmd(
        nc, in_maps, core_ids=list(range(NCORES)), trace=_trace,
        tmpdir=_tmpdir
    )
    out = np.concatenate([res.results[d]["out"] for d in range(NCORES)])
    if _trace:
        kernel.last_results = res
    return out


# revision 18
# speedup vs baseline: 1.8302x; 1.2398x over previous
"""Trainium2 Bass kernel for the PageRank-propagation problem.

out[i] = (1-C) * sum_j P[i,j] * t[j] + C/n
  P = |Bsym - sim|,  Bsym = triu(B,1) + triu(B,1).T
  t[j] = s[j] / rs[j],  rs[j] = sum_k P[j,k]

Sharding: rows split across 8 cores (1024 rows each).  TRANSPOSED tile
layout: each core stores P^T chunks [128 partitions = global column j,
1024 free = local row i], with global columns ROTATED by r0 so the SPMD
program is identical on every core: program chunk jc covers global
columns (r0 + jc*128 + p) mod n.  Chunks 0..7 are the diagonal band.

Inputs are downcast to fp8 e4m3 on the host (quarter HBM traffic vs
f32; the kernel is memory-bound).  P is materialized in bf16 in SBUF.

Phase 1, per chunk, split across engines to stay under the DMA roofline:
  - PE path:  D = I*X + (-I)*sim into PSUM (fp8 matmuls), ACT Abs
    PSUM->SBUF bf16.  Band chunks take this path after two gpsimd
    affine_selects build the triangular Bsym piece.
  - DVE path: D = X - sim (TT fp8, 1x) into P, then abs in place
    (tensor_scalar abs_max, 4x) or on ACT.
  - rs: PE matmul with a ones vector (partition-dim reduction),
    accumulated across all 64 chunks in PSUM [1, 1024].
AllGather of the per-core row sums (4 KiB -> 32 KiB).  Phase 2:
t = s * recip(rs) computed in a rotated [64,128] layout (contiguous
DMA from a duplicated DRAM copy), PE-transposed to stationary layout
[128, 64]; the GEMV y = P^T.T @ t runs as 128 PE matmuls with t chunks
stationary, accumulating in PSUM [1, 1024]; final affine on ACT.
"""

import sys

sys.path.insert(0, "/opt/trn_rl_repo")

import numpy as np

N = 8192
NCORES = 8
NB = N // NCORES          # rows per core (1024)
CW = 128                  # chunk width (columns per chunk = partitions)
NCH = N // CW             # 64 chunks per core
BCH = NB // CW            # 8 band chunks
OBCH = NCH - BCH          # 56 off-band chunks
G = 4                     # off-band chunks per DMA group
NOG = OBCH // G           # 14 off-band DMA groups
BG = 2                    # band chunks per DMA group
NBG = BCH // BG           # 4 band DMA groups
C = 0.15

# engine split for off-band chunks (tunable)
PE_OB = 24                # off-band chunks subtracted on PE (rest on DVE)
ACT_STEAL = 4             # every ACT_STEAL-th DVE chunk's abs goes to ACT
TSCALE = 4096.0           # t is scaled into fp8 range; undone in final affine

_built = {}


def _ob_assign():
    """Assignment of the 56 off-band chunks: True = PE path."""
    pe = []
    cnt = 0
    for i in range(OBCH):
        want = (PE_OB * (i + 1)) // OBCH
        pe.append(want > cnt)
        cnt = want
    return pe


def _build():
    if "nc" in _built:
        return _built["nc"]
    import concourse.bass as bass
    import concourse.bacc as bacc
    import concourse.tile as tile
    from concourse import mybir

    dt = mybir.dt
    Alu = mybir.AluOpType
    Act = mybir.ActivationFunctionType

    nc = bacc.Bacc(
        "TRN2", target_bir_lowering=False, debug=False, enable_asserts=False,
        num_devices=NCORES,
    )

    # off-band groups: [group, partition, chunk-in-group * (X | simT)]
    OB = nc.dram_tensor("ob", [NOG, 128, G, 2 * NB], dt.float8e4,
                        kind="ExternalInput")
    # band groups: [group, partition, chunk-in-group * (M1 | M2 | simT)]
    BND = nc.dram_tensor("bnd", [NBG, 128, BG, 3 * NB], dt.float8e4,
                         kind="ExternalInput")
    # s rotated: srot[c, p] = s[(r0 + c*128 + p) % N]
    SROT = nc.dram_tensor("srot", [NCH, CW], dt.float32, kind="ExternalInput")
    OUT = nc.dram_tensor("out", [NB], dt.float32, kind="ExternalOutput")

    pe_ob = _ob_assign()

    with tile.TileContext(nc, num_cores=NCORES) as tc:
        import contextlib

        with contextlib.ExitStack() as ctx:
            pp = ctx.enter_context(tc.tile_pool(name="pp", bufs=1))
            constp = ctx.enter_context(tc.tile_pool(name="constp", bufs=1))
            statp = ctx.enter_context(tc.tile_pool(name="statp", bufs=1))
            dramp = ctx.enter_context(
                tc.tile_pool(name="dramp", bufs=1, space="DRAM")
            )
            rspp = ctx.enter_context(
                tc.tile_pool(name="rspp", bufs=1, space="PSUM")
            )

            # ---- persistent tiles ----
            # P^T in fp8; two halves
            P_sba = pp.tile([128, (NCH // 2) * NB], dt.float8e4)
            P_sbb = pp.tile([128, (NCH // 2) * NB], dt.float8e4)

            id_pos = constp.tile([128, 128], dt.float8e4)
            id_neg = constp.tile([128, 128], dt.float8e4)
            id64 = constp.tile([64, 64], dt.bfloat16)
            # DoubleRow lhsT needs its two planes >=16B apart
            ones_f8 = constp.tile([128, 32], dt.float8e4)

            srot_sb = statp.tile([NCH, CW], dt.float32)
            rs_rot = statp.tile([NCH, CW], dt.bfloat16)
            trc = statp.tile([NCH, CW], dt.float32)
            t_rot_bf = statp.tile([NCH, CW], dt.bfloat16)
            t_stat = statp.tile([128, NCH], dt.float8e4)
            rs_sb = statp.tile([1, NB], dt.bfloat16)
            out_sb = statp.tile([1, NB], dt.float32)

            cc_in = dramp.tile([NB], dt.bfloat16)
            cc_out = dramp.tile([N], dt.bfloat16, addr_space="Shared")
            cc_dup = dramp.tile([2 * N], dt.bfloat16)

            # rs accumulator: [1, 1024] f32 = 2 PSUM banks, pinned all of
            # phase 1
            rs_ps = rspp.tile([1, NB], dt.float32)

            def P_chunk(jc, h0, h1):
                """P slice for chunk jc, free cols [h0:h1) of its 1024."""
                half = P_sba if jc < NCH // 2 else P_sbb
                base = (jc % (NCH // 2)) * NB
                return half[:, base + h0: base + h1]

            # ---- constants ----
            nc.gpsimd.memset(id_pos[:], 0.0)
            nc.gpsimd.affine_select(
                out=id_pos[:], in_=id_pos[:], compare_op=Alu.not_equal,
                fill=1.0, base=0, channel_multiplier=1, pattern=[[-1, 128]],
            )
            nc.gpsimd.memset(id_neg[:], 0.0)
            nc.gpsimd.affine_select(
                out=id_neg[:], in_=id_neg[:], compare_op=Alu.not_equal,
                fill=-1.0, base=0, channel_multiplier=1, pattern=[[-1, 128]],
            )
            nc.gpsimd.memset(id64[:], 0.0)
            nc.gpsimd.affine_select(
                out=id64[:], in_=id64[:], compare_op=Alu.not_equal,
                fill=1.0, base=0, channel_multiplier=1, pattern=[[-1, 64]],
            )
            nc.vector.memset(ones_f8[:], 1.0)
            nc.sync.dma_start(out=srot_sb[:], in_=SROT.ap())

            # [128, 2, 1] with 16-element plane stride
            ones_dr = ones_f8[:].rearrange("p (a m) -> p a m", a=2)[:, :, 0:1]
            rs_n = [0]
            chunk_done = set()

            def P_pair(jp, h):
                """DoubleRow moving AP: chunks (2jp, 2jp+1), cols [h:h+512)."""
                half = P_sba if 2 * jp < NCH // 2 else P_sbb
                base = ((2 * jp) % (NCH // 2)) * NB
                v = half[:, base: base + 2 * NB].rearrange(
                    "p (two n) -> p two n", two=2
                )
                return v[:, :, h:h + 512]

            def rs_mark(jc):
                """Chunk jc finished; emit DoubleRow row-sum MMs for complete
                chunk pairs."""
                chunk_done.add(jc)
                jp = jc // 2
                if (2 * jp in chunk_done) and (2 * jp + 1 in chunk_done):
                    rs_n[0] += 1
                    first = rs_n[0] == 1
                    last = rs_n[0] == NCH // 2
                    for h in (0, 512):
                        nc.tensor.matmul(
                            rs_ps[:, h:h + 512], ones_dr, P_pair(jp, h),
                            start=first, stop=last,
                            perf_mode=mybir.MatmulPerfMode.DoubleRow,
                        )

            # ---- phase 1 ----
            with contextlib.ExitStack() as p1:
                obp = p1.enter_context(tc.tile_pool(name="obp", bufs=3))
                bndp = p1.enter_context(tc.tile_pool(name="bndp", bufs=2))
                xselp = p1.enter_context(tc.tile_pool(name="xselp", bufs=4))
                psp = p1.enter_context(
                    tc.tile_pool(name="psp", bufs=2, space="PSUM")
                )

                dve_i = [0]

                def do_ob_chunk(jc, xt, a):
                    """Off-band chunk jc from slot a of group tile xt."""
                    x_lo = xt[:, a, 0:NB]
                    s_lo = xt[:, a, NB:2 * NB]
                    if pe_ob[jc - BCH]:
                        ps = psp.tile([128, NB], dt.float32, tag="ps")
                        for h in (0, 512):
                            nc.tensor.matmul(
                                ps[:, h:h + 512], id_pos[:],
                                xt[:, a, h:h + 512],
                                start=True, stop=False,
                            )
                        for h in (0, 512):
                            nc.tensor.matmul(
                                ps[:, h:h + 512], id_neg[:],
                                xt[:, a, NB + h:NB + h + 512],
                                start=False, stop=True,
                            )
                        nc.scalar.activation(
                            out=P_chunk(jc, 0, NB), in_=ps[:], func=Act.Abs,
                        )
                    else:
                        pc = P_chunk(jc, 0, NB)
                        nc.vector.tensor_tensor(
                            out=pc, in0=x_lo, in1=s_lo, op=Alu.subtract,
                        )
                        if dve_i[0] % ACT_STEAL == 0:
                            nc.scalar.activation(out=pc, in_=pc, func=Act.Abs)
                        else:
                            # abs via sign-bit clear (uint8 view, 2x mode)
                            pcu = pc.bitcast(dt.uint8)
                            nc.vector.tensor_scalar(
                                out=pcu, in0=pcu, scalar1=0x7F,
                                scalar2=None, op0=Alu.bitwise_and,
                            )
                        dve_i[0] += 1
                    rs_mark(jc)

                def do_band_chunk(jc, bt, a):
                    """Band chunk jc from slot a of band group tile bt.

                    Only the 128 columns il in [jc*128, jc*128+128) straddle
                    the diagonal; left of that X = M1, right X = M2.  The two
                    affine selects shrink to [128, 128]."""
                    m0 = jc * CW          # mid start
                    xs1 = xselp.tile([128, CW], dt.float8e4, tag="xs")
                    xs2 = xselp.tile([128, CW], dt.float8e4, tag="xs")
                    # keep M1_mid where (il - m0) < p
                    nc.gpsimd.affine_select(
                        out=xs1[:], in_=bt[:, a, m0:m0 + CW],
                        compare_op=Alu.is_gt,
                        fill=0.0, base=0, channel_multiplier=1,
                        pattern=[[-1, CW]],
                    )
                    # keep M2_mid where (il - m0) > p
                    nc.gpsimd.affine_select(
                        out=xs2[:], in_=bt[:, a, NB + m0:NB + m0 + CW],
                        compare_op=Alu.is_gt,
                        fill=0.0, base=0, channel_multiplier=-1,
                        pattern=[[1, CW]],
                    )
                    ps = psp.tile([128, NB], dt.float32, tag="ps")
                    for h in (0, 512):
                        # -sim initializes the whole bank (one start=True
                        # per 2 KiB PSUM zero-region), X segments accumulate
                        segs = []
                        lo, hi = h, min(m0, h + 512)      # left: X = M1
                        if hi > lo:
                            segs.append((lo, hi, bt[:, a, lo:hi]))
                        if h <= m0 < h + 512:             # mid: selects
                            segs.append((m0, m0 + CW, xs1[:]))
                            segs.append((m0, m0 + CW, xs2[:]))
                        lo, hi = max(m0 + CW, h), h + 512  # right: X = M2
                        if hi > lo:
                            segs.append((lo, hi, bt[:, a, NB + lo:NB + hi]))
                        nc.tensor.matmul(
                            ps[:, h:h + 512], id_neg[:],
                            bt[:, a, 2 * NB + h:2 * NB + h + 512],
                            start=True, stop=False,
                        )
                        for k, (lo, hi, src) in enumerate(segs):
                            nc.tensor.matmul(
                                ps[:, lo:hi], id_pos[:], src,
                                start=False, stop=(k == len(segs) - 1),
                            )
                    nc.scalar.activation(
                        out=P_chunk(jc, 0, NB), in_=ps[:], func=Act.Abs,
                    )
                    rs_mark(jc)

                bg_done = 0
                for g in range(NOG):
                    xt = obp.tile([128, G, 2 * NB], dt.float8e4, tag="ob")
                    nc.sync.dma_start(out=xt[:], in_=OB[g])
                    for a in range(G):
                        do_ob_chunk(BCH + g * G + a, xt, a)
                    # interleave band groups through phase 1
                    if g % 3 == 2 and bg_done < NBG:
                        bt = bndp.tile([128, BG, 3 * NB], dt.float8e4,
                                       tag="bnd")
                        nc.sync.dma_start(out=bt[:], in_=BND[bg_done])
                        for a in range(BG):
                            do_band_chunk(bg_done * BG + a, bt, a)
                        bg_done += 1
                while bg_done < NBG:
                    bt = bndp.tile([128, BG, 3 * NB], dt.float8e4, tag="bnd")
                    nc.sync.dma_start(out=bt[:], in_=BND[bg_done])
                    for a in range(BG):
                        do_band_chunk(bg_done * BG + a, bt, a)
                    bg_done += 1

            # ---- row sums -> AllGather (natural order) ----
            nc.scalar.activation(out=rs_sb[:], in_=rs_ps[:], func=Act.Copy)
            nc.sync.dma_start(
                out=cc_in[:].rearrange("(a f) -> a f", a=1), in_=rs_sb[:]
            )
            nc.gpsimd.collective_compute(
                "AllGather", Alu.bypass,
                replica_groups=[list(range(NCORES))],
                ins=[cc_in[:]], outs=[cc_out[:]],
            )
            nc.sync.dma_start(out=cc_dup[0:N], in_=cc_out[:])
            nc.sync.dma_start(out=cc_dup[N:2 * N], in_=cc_out[:])

            r0v = nc.partition_id() * NB

            # ---- t = s * recip(rs), rotated, to stationary layout ----
            nc.sync.dma_start(
                out=rs_rot[:],
                in_=cc_dup[bass.ds(r0v, N)].rearrange("(c p) -> c p", c=NCH),
            )
            nc.vector.reciprocal(out=trc[:], in_=rs_rot[:])
            nc.vector.tensor_tensor(
                out=trc[:], in0=trc[:], in1=srot_sb[:], op=Alu.mult
            )
            # scale t into fp8 range (undone in the final affine)
            nc.scalar.activation(
                out=t_rot_bf[:], in_=trc[:], func=Act.Copy, scale=TSCALE,
            )

            with contextlib.ExitStack() as p2:
                psp2 = p2.enter_context(
                    tc.tile_pool(name="psp2", bufs=1, space="PSUM")
                )
                tp_ps = psp2.tile([128, NCH], dt.bfloat16)
                y_ps = psp2.tile([1, NB], dt.float32)

                # transpose [64, 128] -> [128, 64]
                nc.tensor.transpose(tp_ps[:], t_rot_bf[:], id64[:])
                # parity-split copy: t_stat[p, a*32 + jp] = t[chunk 2jp+a]
                # (DoubleRow lhsT planes must be >=16B apart)
                nc.vector.tensor_copy(
                    out=t_stat[:].rearrange("p (a jp) -> p a jp", a=2),
                    in_=tp_ps[:].rearrange("p (jp a) -> p a jp", a=2),
                )

                # ---- phase 2: GEMV y[i] = sum_j P^T[j, i] t[j] on PE ----
                # DoubleRow: chunk pairs, t pair stationary [128, 2, 1]
                t_split = t_stat[:].rearrange("p (a jp) -> p a jp", a=2)
                for jp in range(NCH // 2):
                    t_dr = t_split[:, :, jp:jp + 1]
                    for h in (0, 512):
                        nc.tensor.matmul(
                            y_ps[:, h:h + 512], t_dr, P_pair(jp, h),
                            start=(jp == 0), stop=(jp == NCH // 2 - 1),
                            perf_mode=mybir.MatmulPerfMode.DoubleRow,
                        )

                # out = (1-C)/TSCALE * y' + C/n
                nc.scalar.activation(
                    out=out_sb[:], in_=y_ps[:], func=Act.Copy,
                    bias=float(C / N), scale=float((1.0 - C) / TSCALE),
                )
            nc.sync.dma_start(
                out=OUT.ap().rearrange("(a f) -> a f", a=1), in_=out_sb[:]
            )

    nc.finalize()
    _built["nc"] = nc
    return nc


def _shard_inputs(B, sim, s):
    """Layout-only host transforms (slice / transpose / concat / pack),
    plus an fp8 downcast (precision choice of the sharding format)."""
    import ml_dtypes

    f8 = ml_dtypes.float8_e4m3
    B8 = B.astype(f8)
    sim8 = sim.astype(f8)
    s_ext = np.concatenate([s, s]).astype(np.float32)

    in_maps = []
    for d in range(NCORES):
        r0, r1 = d * NB, (d + 1) * NB

        # off-band chunks, rotated: program chunk jc (8..63) covers global
        # columns j0 = (r0 + jc*128) % N .. +128
        ob = np.empty((NOG, 128, G, 2 * NB), dtype=f8)
        for i in range(OBCH):
            jc = BCH + i
            j0 = (r0 + jc * CW) % N
            if j0 >= r1:
                x = np.ascontiguousarray(B8[r0:r1, j0:j0 + CW].T)
            else:
                x = B8[j0:j0 + CW, r0:r1]
            st = np.ascontiguousarray(sim8[r0:r1, j0:j0 + CW].T)
            g, a = i // G, i % G
            ob[g, :, a, 0:NB] = x
            ob[g, :, a, NB:] = st

        # band chunks: global columns inside [r0, r1)
        Bblk = B8[r0:r1, r0:r1]
        sblk = sim8[r0:r1, r0:r1]
        bnd = np.empty((NBG, 128, BG, 3 * NB), dtype=f8)
        for jc in range(BCH):
            cs = slice(jc * CW, (jc + 1) * CW)
            g, a = jc // BG, jc % BG
            bnd[g, :, a, 0:NB] = np.ascontiguousarray(Bblk[:, cs].T)
            bnd[g, :, a, NB:2 * NB] = Bblk[cs, :]
            bnd[g, :, a, 2 * NB:] = np.ascontiguousarray(sblk[:, cs].T)

        srot = np.ascontiguousarray(
            s_ext[r0:r0 + N].reshape(NCH, CW)
        )

        in_maps.append({
            "ob": ob,
            "bnd": bnd,
            "srot": srot,
        })
    return in_maps


def kernel(B, similarity_matrix, connectivity_scores, _trace=False,
           _tmpdir=None):
    from concourse import bass_utils

    B = np.asarray(B, dtype=np.float32)
    sim = np.asarray(similarity_matrix, dtype=np.float32)
    s = np.asarray(connectivity_scores, dtype=np.float32)

    nc = _build()
    in_maps = _shard_inputs(B, sim, s)
    res = bass_utils.run_bass_kernel_spmd(
        nc, in_maps, core_ids=list(range(NCORES)), trace=_trace,
        tmpdir=_tmpdir
    )
    out = np.concatenate([res.results[d]["out"] for d in range(NCORES)])
    if _trace:
        kernel.last_results = res
    return out
